# revision 1
# baseline (speedup 1.0000x reference)
"""MultiHeadAttention Trainium2 kernel (8 NeuronCores, data-parallel over batch).

Contract: kernel(**inputs) takes the FULL inputs from setup_inputs() and
returns the FULL [8, 512, 1024] output. Internally, batch element c goes to
NeuronCore c (B == n_cores == 8); each core runs the same Bass/Tile program
on its own shard. No collectives needed.

Per-core computation (batch b, S=512, D=1024, H=16, Dk=64):
  QT = (w_q/8)^T-proj of query^T  -> [D, S]  (head h rows h*64..h*64+63)
  KT likewise (unscaled)          -> [D, S]
  V  = natural value proj         -> [S, D]  (stored with a ones column per head)
  per head: scoresT[k,q'] = KT_h-block^T @ QT_h   (q' = reversed query index)
            psum += amt[h]  (host-precomputed fp16 rel-bias + mask additive)
            attnT = exp(psum)                     (ScalarE, reads PSUM)
            ctxT[65,S] = [V_h | 1]^T @ attnT      (row 64 = softmax denominators)
            ctxT_norm = ctxT[0:64] * broadcast(1/denom)
  out_rev[q', e] = ctxT_norm^T-chunks @ w_o^T + b_o ; host un-reverses rows.

All matmuls run in fp32r (single-pass, 4x faster than fp32 LOW_HIGH mode);
matmul operands are pre-rounded to fp32r's 19-bit mantissa on the host (DMA
inputs) or rounded on write by ACT/DVE (on-chip producers), which the BIR
verifier requires. The query-direction reversal makes the relative-position
bias rel_bias[k - q + 511, h] equal rel_bias[k + q', h] — a positive-stride
layout the host can materialize directly.
"""
import numpy as np

import concourse.bass as bass
import concourse.tile as tile
from concourse import bacc, mybir
from concourse.bass_utils import run_bass_kernel_spmd

S = 512
D = 1024
H = 16
DK = 64
N_CORES = 8
NCH = D // 128  # 8 d-model chunks of 128
SB = S // 128   # 4 seq blocks of 128
F32 = mybir.dt.float32
F32R = mybir.dt.float32r
F16 = mybir.dt.float16

MASK_NEG = -30000.0  # large-negative additive mask, fp16-representable

_CACHE = {}


def _build_program():
    nc = bacc.Bacc("TRN2", target_bir_lowering=False, debug=False,
                   num_devices=N_CORES)

    # Per-core DRAM inputs (fp32r ones feed matmuls; host pre-rounds them)
    qT = nc.dram_tensor("qT", [D, S], F32R, kind="ExternalInput").ap()
    kT = nc.dram_tensor("kT", [D, S], F32R, kind="ExternalInput").ap()
    vT = nc.dram_tensor("vT", [D, S], F32R, kind="ExternalInput").ap()
    amt = nc.dram_tensor("amt", [H, S, S], F16, kind="ExternalInput").ap()
    wq = nc.dram_tensor("wq", [D, D], F32R, kind="ExternalInput").ap()
    wk = nc.dram_tensor("wk", [D, D], F32R, kind="ExternalInput").ap()
    wv = nc.dram_tensor("wv", [D, D], F32R, kind="ExternalInput").ap()
    wo = nc.dram_tensor("wo", [D, D], F32R, kind="ExternalInput").ap()
    bq = nc.dram_tensor("bq", [128, NCH], F32, kind="ExternalInput").ap()
    bk = nc.dram_tensor("bk", [128, NCH], F32, kind="ExternalInput").ap()
    bvr = nc.dram_tensor("bvr", [1, D], F32R, kind="ExternalInput").ap()
    bor = nc.dram_tensor("bor", [1, D], F32R, kind="ExternalInput").ap()
    out = nc.dram_tensor("out", [S, D], F32, kind="ExternalOutput").ap()

    # DRAM views for chunked DMA
    qT3 = qT.rearrange("(c p) s -> p c s", p=128)     # [128, 8, 512]
    kT3 = kT.rearrange("(c p) s -> p c s", p=128)
    vT3 = vT.rearrange("(c p) s -> p c s", p=128)
    amt4 = amt.rearrange("h (kb p) q -> h p kb q", p=128)  # [16, 128, 4, 512]
    wq3 = wq.rearrange("(c p) e -> c p e", p=128)     # [8, 128, 1024]
    wk3 = wk.rearrange("(c p) e -> c p e", p=128)
    wv3 = wv.rearrange("(c p) e -> c p e", p=128)
    wo3 = wo.rearrange("(c p) e -> c p e", p=128)
    out3 = out.rearrange("(sb p) e -> sb p e", p=128)  # [4, 128, 1024]

    from contextlib import ExitStack

    with tile.TileContext(nc) as tc, ExitStack() as ctx:
        singles = ctx.enter_context(tc.tile_pool(name="singles", bufs=1))
        wpool = ctx.enter_context(tc.tile_pool(name="wpool", bufs=16))
        rcpool = ctx.enter_context(tc.tile_pool(name="rcpool", bufs=2))
        rc1pool = ctx.enter_context(tc.tile_pool(name="rc1pool", bufs=1))
        ps_proj = ctx.enter_context(tc.tile_pool(name="ps_proj", bufs=2, space="PSUM"))
        ps_sc = ctx.enter_context(tc.tile_pool(name="ps_sc", bufs=3, space="PSUM"))
        ps_ctx = ctx.enter_context(tc.tile_pool(name="ps_ctx", bufs=2, space="PSUM"))
        ps_r = ctx.enter_context(tc.tile_pool(name="ps_r", bufs=1, space="PSUM"))
        vt_ctx = ExitStack()
        vtpool = vt_ctx.enter_context(tc.tile_pool(name="vtpool", bufs=1))

        # small constants first so the warm-up matmuls can start immediately
        bq_sb = singles.tile([128, NCH], F32, tag="bq")
        bk_sb = singles.tile([128, NCH], F32, tag="bk")
        nc.sync.dma_start(out=bq_sb, in_=bq)
        nc.sync.dma_start(out=bk_sb, in_=bk)
        bvr_sb = singles.tile([1, D], F32R, tag="bvr")
        bor_sb = singles.tile([1, D], F32R, tag="bor")
        nc.sync.dma_start(out=bvr_sb, in_=bvr)
        nc.sync.dma_start(out=bor_sb, in_=bor)
        # memset can't target fp32r; stage in fp32 and round via ACT copy
        ones_f32 = singles.tile([1, 128], F32, tag="ones_f32")
        nc.vector.memset(ones_f32, 1.0)
        ones_sb = singles.tile([1, 128], F32R, tag="ones")
        nc.scalar.copy(ones_sb, ones_f32)
        ones_col = singles.tile([128, H, 1], F32, tag="ones_col")
        nc.vector.memset(ones_col, 1.0)

        # HAM warm-up: ~5us of throwaway matmuls while the input DMAs stream,
        # so the PE clock-gate is at 8/8 by the time real work is ready.
        # Operands are built on-chip so no DMA gates the first matmul.
        for _ in range(40):
            pd = ps_proj.tile([128, 512], F32, tag="proj")
            nc.tensor.matmul(pd[:, :128], lhsT=ones_sb[:, :128], rhs=ones_sb,
                             start=True, stop=True)

        # bulk loads, emitted in consumption order (sync queue is FIFO)
        vT_sb = vtpool.tile([128, NCH, S], F32R, tag="vT")
        nc.sync.dma_start(out=vT_sb, in_=vT3)
        qT_sb = singles.tile([128, NCH, S], F32R, tag="qT")
        kT_sb = singles.tile([128, NCH, S], F32R, tag="kT")

        # big persistent activations
        QT_sb = singles.tile([128, NCH, S], F32R, tag="QT")
        KT_sb = singles.tile([128, NCH, S], F32R, tag="KT")
        # V with a ones column appended per head: [128, sb, 16*65]
        V_sb = singles.tile([128, SB, H * (DK + 1)], F32R, tag="V")
        ctxT_sb = singles.tile([128, NCH, S], F32R, tag="ctxT")

        # ---- V projection: V[s, e] = vT^T @ wvT + b_v ----
        wv_sb = []
        for dc in range(NCH):
            t = wpool.tile([128, D], F32R, tag="w")
            nc.sync.dma_start(out=t, in_=wv3[dc])
            wv_sb.append(t)
        nc.sync.dma_start(out=qT_sb, in_=qT3)
        nc.sync.dma_start(out=kT_sb, in_=kT3)
        for sb in range(SB):
            # set ones columns for this s-block
            v_heads = V_sb[:, sb, :].rearrange("p (h c) -> p h c", c=DK + 1)
            nc.scalar.copy(v_heads[:, :, DK:DK + 1], ones_col)
            for eh in range(2):
                pv = ps_proj.tile([128, 512], F32, tag="proj")
                for dc in range(NCH):
                    nc.tensor.matmul(
                        pv,
                        lhsT=vT_sb[:, dc, sb * 128:(sb + 1) * 128],
                        rhs=wv_sb[dc][:, eh * 512:(eh + 1) * 512],
                        start=(dc == 0), stop=False,
                    )
                nc.tensor.matmul(
                    pv, lhsT=ones_sb[:, :128],
                    rhs=bvr_sb[:, eh * 512:(eh + 1) * 512],
                    start=False, stop=True,
                )
                nc.scalar.copy(
                    v_heads[:, 8 * eh:8 * eh + 8, 0:DK],
                    pv.rearrange("p (h d) -> p h d", d=DK),
                )

        # vT no longer needed; free its SBUF for the pools below
        vt_ctx.close()
        amtpool = ctx.enter_context(tc.tile_pool(name="amtpool", bufs=2))
        attnpool = ctx.enter_context(tc.tile_pool(name="attnpool", bufs=9))
        outpool = ctx.enter_context(tc.tile_pool(name="outpool", bufs=2))

        # ---- interleaved Q/K projection chunks + attention heads ----
        wq_sb = []
        wk_sb = []
        for dc in range(NCH):
            t = wpool.tile([128, D], F32R, tag="w")
            nc.sync.dma_start(out=t, in_=wq3[dc])
            wq_sb.append(t)
        for dc in range(NCH):
            t = wpool.tile([128, D], F32R, tag="w")
            nc.sync.dma_start(out=t, in_=wk3[dc])
            wk_sb.append(t)
        # wo prefetch: emitted here so it sits early on the sync queue; its
        # slot-wait resolves as soon as the wv tiles retire after V-proj
        wo_sb = []
        for ch in range(NCH):
            t = wpool.tile([128, D], F32R, tag="w")
            nc.sync.dma_start(out=t, in_=wo3[ch])
            wo_sb.append(t)

        # Software pipeline over heads: head h's context matmuls are emitted
        # one head later, so PE fills the DVE-add -> ACT-exp latency of head
        # h with head h+1's scores (and the next chunk's projections) and
        # never idles long enough for HAM to re-throttle.
        def emit_scores(h):
            i, p0 = h // 2, (h % 2) * 64
            amt_h = amtpool.tile([128, SB, S], F16, tag="amt")
            # GpSimd (SWDGE) queue: keeps amt streams off the sync queue so
            # weight prefetch (esp. wo) isn't stuck behind them, and off the
            # busy compute engines' FIFOs (GpSimd is otherwise nearly idle)
            nc.gpsimd.dma_start(out=amt_h, in_=amt4[h])
            QT_h = QT_sb[p0:p0 + 64, i, :]
            attn_tiles = []
            for kb in range(SB):
                ps = ps_sc.tile([128, 512], F32, tag="sc")
                nc.tensor.matmul(
                    ps, lhsT=KT_sb[p0:p0 + 64, i, kb * 128:(kb + 1) * 128],
                    rhs=QT_h, start=True, stop=True,
                )
                nc.vector.tensor_add(ps, ps, amt_h[:, kb, :])
                at = attnpool.tile([128, 512], F32R, tag="attn")
                nc.scalar.activation(at, ps, mybir.ActivationFunctionType.Exp)
                attn_tiles.append(at)
            return attn_tiles

        def emit_ctx(h, attn_tiles):
            i, p0 = h // 2, (h % 2) * 64
            pc = ps_ctx.tile([DK + 1, 512], F32, tag="ctx")
            for kb in range(SB):
                nc.tensor.matmul(
                    pc, lhsT=V_sb[:, kb, h * 65:(h + 1) * 65],
                    rhs=attn_tiles[kb], start=(kb == 0), stop=(kb == SB - 1),
                )
            # custom-DVE reciprocal can't read PSUM on HW; stage sums in SBUF
            sums_sb = rcpool.tile([1, 512], F32, tag="recip")
            nc.scalar.copy(sums_sb, pc[DK:DK + 1, :])
            recip_f32 = rcpool.tile([1, 512], F32, tag="recip")
            nc.vector.reciprocal_approx_fast(out=recip_f32, in_=sums_sb)
            recip = rc1pool.tile([1, 512], F32R, tag="recip_r")
            nc.scalar.copy(recip, recip_f32)
            # broadcast 1/denom across 64 partitions via a K=1 matmul
            # (GpSimd's queue is reserved for the amt DMA stream)
            pr = ps_r.tile([64, 512], F32, tag="r")
            nc.tensor.matmul(pr, lhsT=ones_sb[:, :64], rhs=recip,
                             start=True, stop=True)
            r_sb = rc1pool.tile([64, 512], F32, tag="rbc")
            nc.scalar.copy(r_sb, pr)
            nc.vector.tensor_mul(ctxT_sb[p0:p0 + 64, i, :], pc[0:DK, :], r_sb)

        pending = None  # (head, attn_tiles) awaiting its context matmuls
        for i in range(NCH):  # e-chunk i covers heads 2i, 2i+1
            pq = ps_proj.tile([128, 512], F32, tag="proj")
            for dc in range(NCH):
                nc.tensor.matmul(
                    pq, lhsT=wq_sb[dc][:, i * 128:(i + 1) * 128],
                    rhs=qT_sb[:, dc, :],
                    start=(dc == 0), stop=(dc == NCH - 1),
                )
            nc.scalar.add(QT_sb[:, i, :], pq, bq_sb[:, i:i + 1])
            pk = ps_proj.tile([128, 512], F32, tag="proj")
            for dc in range(NCH):
                nc.tensor.matmul(
                    pk, lhsT=wk_sb[dc][:, i * 128:(i + 1) * 128],
                    rhs=kT_sb[:, dc, :],
                    start=(dc == 0), stop=(dc == NCH - 1),
                )
            nc.scalar.add(KT_sb[:, i, :], pk, bk_sb[:, i:i + 1])

            for sub in range(2):
                h = 2 * i + sub
                tiles = emit_scores(h)
                if pending is not None:
                    emit_ctx(*pending)
                pending = (h, tiles)
        emit_ctx(*pending)

        # ---- output projection: out_rev[q', e] = ctxT^T @ woT + b_o ----
        for sb in range(SB):
            for eh in range(2):
                po = ps_proj.tile([128, 512], F32, tag="proj")
                for ch in range(NCH):
                    nc.tensor.matmul(
                        po, lhsT=ctxT_sb[:, ch, sb * 128:(sb + 1) * 128],
                        rhs=wo_sb[ch][:, eh * 512:(eh + 1) * 512],
                        start=(ch == 0), stop=False,
                    )
                nc.tensor.matmul(
                    po, lhsT=ones_sb[:, :128],
                    rhs=bor_sb[:, eh * 512:(eh + 1) * 512],
                    start=False, stop=True,
                )
                osb = outpool.tile([128, 512], F32, tag="out")
                nc.scalar.copy(osb, po)
                nc.sync.dma_start(
                    out=out3[sb, :, eh * 512:(eh + 1) * 512], in_=osb)

    nc.compile()
    return nc


def _round_f32r(a):
    """Round-to-nearest to fp32r's 19-bit (1+8+13... wait — explicit 13-bit)
    mantissa so the PE's fp32r truncation is lossless on these operands."""
    b = np.ascontiguousarray(a, np.float32).view(np.uint32)
    return ((b + 0x1000) & np.uint32(0xFFFFE000)).view(np.float32)


def _prep_inputs(query, key, value, mask, w_q, b_q, w_k, b_k, w_v, b_v,
                 w_o, b_o, rel_bias):
    query = np.asarray(query, np.float32)
    key = np.asarray(key, np.float32)
    value = np.asarray(value, np.float32)
    mask = np.asarray(mask)
    w_q = np.asarray(w_q, np.float32)
    w_k = np.asarray(w_k, np.float32)
    w_v = np.asarray(w_v, np.float32)
    w_o = np.asarray(w_o, np.float32)
    b_q = np.asarray(b_q, np.float32)
    b_k = np.asarray(b_k, np.float32)
    b_v = np.asarray(b_v, np.float32)
    b_o = np.asarray(b_o, np.float32)
    rel_bias = np.asarray(rel_bias, np.float32)

    shared = {
        "wq": _round_f32r(w_q.T / 8.0),
        "wk": _round_f32r(w_k.T),
        "wv": _round_f32r(w_v.T),
        "wo": _round_f32r(w_o.T),
        "bq": np.ascontiguousarray((b_q / 8.0).reshape(NCH, 128).T),
        "bk": np.ascontiguousarray(b_k.reshape(NCH, 128).T),
        "bvr": _round_f32r(b_v.reshape(1, D)),
        "bor": _round_f32r(b_o.reshape(1, D)),
    }

    # biasT_rev[h, k, q'] = rel_bias[k + q', h]
    idx = np.arange(S)[:, None] + np.arange(S)[None, :]  # [k, q'] in [0, 1022]
    bias_t = rel_bias[idx]                 # [S, S, H]
    bias_t = np.ascontiguousarray(bias_t.transpose(2, 0, 1))  # [H, k, q']

    in_maps = []
    for c in range(N_CORES):
        # maskT_rev[k, q'] additive: mask[c, 0, 511-q', k] == 0 -> MASK_NEG
        m = mask[c, 0][::-1, :].T          # [k, q'] values in {0, 1}
        madd = np.where(m == 0, np.float32(MASK_NEG), np.float32(0.0))
        amt = (bias_t + madd[None]).astype(np.float16)
        im = dict(shared)
        im["qT"] = _round_f32r(query[c].T[:, ::-1])
        im["kT"] = _round_f32r(key[c].T)
        im["vT"] = _round_f32r(value[c].T)
        im["amt"] = np.ascontiguousarray(amt)
        in_maps.append(im)
    return in_maps


def kernel(query, key, value, mask, w_q, b_q, w_k, b_k, w_v, b_v, w_o, b_o,
           rel_bias, _run_opts=None):
    if "nc" not in _CACHE:
        _CACHE["nc"] = _build_program()
    nc = _CACHE["nc"]
    in_maps = _prep_inputs(query, key, value, mask, w_q, b_q, w_k, b_k,
                           w_v, b_v, w_o, b_o, rel_bias)
    opts = _run_opts or {}
    res = run_bass_kernel_spmd(nc, in_maps, list(range(N_CORES)), **opts)
    out = np.stack([res.results[c]["out"][::-1, :] for c in range(N_CORES)])
    if _run_opts is not None:
        _CACHE["last_result"] = res
    return out.astype(np.float32)



# revision 11
# speedup vs baseline: 1.3369x; 1.3369x over previous
"""MultiHeadAttention Trainium2 kernel (8 NeuronCores, data-parallel over batch).

Contract: kernel(**inputs) takes the FULL inputs from setup_inputs() and
returns the FULL [8, 512, 1024] output. Batch element c runs on NeuronCore c
(B == n_cores == 8); each core runs the same Bass/Tile program on its own
shard. No collectives.

All matmuls run in bf16 (full-rate 1 elem/cycle PE streaming; fp32(r) streams
at half rate on TRN2) with fp32 PSUM accumulation. The additive rel-bias +
mask is applied multiplicatively after the exp:
    exp(QK/8 + bias + maskadd) = exp(QK/8) * eamt,   eamt = exp(bias)*mask01
eamt is precomputed on the host in bf16, so the scores path is
    PE matmul -> ACT exp (one op per [128,1024] two-bank psum tile)
    -> DVE bf16 multiply (2x mode)
with no slow fp32-PSUM-side DVE add. Softmax denominators come from a ones
column appended to V per head (row 64 of the ctx psum); reciprocals are
broadcast across partitions with the GpSimd partition_broadcast ucode op.

Per-core computation (batch b, S=512, D=1024, H=16, Dk=64):
  QT = (w_q/8)-proj of query^T  -> [D, S] bf16   (head h rows h*64..h*64+63)
  KT likewise (unscaled)        -> [D, S] bf16
  V  = natural value proj       -> [S, D] bf16 (+ ones column per head)
  per head: scoresT[k,q] in psum; attn = exp(scoresT)*eamt[h]
            ctxT[65,S] = [V_h | 1]^T @ attn ; ctxT_norm = ctxT[0:64] * (1/row64)
  out[q,e] = ctxT_norm^T-chunks @ w_o^T + b_o    (bf16, fp32 psum)
"""
import numpy as np
import ml_dtypes

import concourse.bass as bass
import concourse.tile as tile
from concourse import bacc, mybir
from concourse.bass_utils import run_bass_kernel_spmd

S = 512
D = 1024
H = 16
DK = 64
N_CORES = 8
NCH = D // 128  # 8 d-model chunks of 128
SB = S // 128   # 4 seq blocks of 128
F32 = mybir.dt.float32
BF16 = mybir.dt.bfloat16
NPBF16 = ml_dtypes.bfloat16

MASK_NEG = -30000.0

_CACHE = {}


def _build_program():
    nc = bacc.Bacc("TRN2", target_bir_lowering=False, debug=False,
                   num_devices=N_CORES)

    # Per-core DRAM inputs
    qT = nc.dram_tensor("qT", [D, S], BF16, kind="ExternalInput").ap()
    kT = nc.dram_tensor("kT", [D, S], BF16, kind="ExternalInput").ap()
    vT = nc.dram_tensor("vT", [D, S], BF16, kind="ExternalInput").ap()
    eamt = nc.dram_tensor("eamt", [H, 128, SB * S], BF16,
                          kind="ExternalInput").ap()
    wqc = nc.dram_tensor("wqc", [128, NCH, D], BF16, kind="ExternalInput").ap()
    wkc = nc.dram_tensor("wkc", [128, NCH, D], BF16, kind="ExternalInput").ap()
    wvc = nc.dram_tensor("wvc", [128, 2, NCH * 512], BF16,
                         kind="ExternalInput").ap()
    woc = nc.dram_tensor("woc", [128, NCH, D], BF16, kind="ExternalInput").ap()
    bqk = nc.dram_tensor("bqk", [128, 2 * NCH], F32, kind="ExternalInput").ap()
    bvo = nc.dram_tensor("bvo", [1, 2 * D], BF16, kind="ExternalInput").ap()
    out = nc.dram_tensor("out", [S, D], BF16, kind="ExternalOutput").ap()

    out3 = out.rearrange("(sb p) e -> sb p e", p=128)  # [4, 128, 1024]

    from contextlib import ExitStack

    with tile.TileContext(nc) as tc, ExitStack() as ctx:
        singles = ctx.enter_context(tc.tile_pool(name="singles", bufs=1))
        eamtpool = ctx.enter_context(tc.tile_pool(name="eamtpool", bufs=4))
        espool = ctx.enter_context(tc.tile_pool(name="espool", bufs=3))
        attnpool = ctx.enter_context(tc.tile_pool(name="attnpool", bufs=4))
        sumspool = ctx.enter_context(tc.tile_pool(name="sumspool", bufs=2))
        recippool = ctx.enter_context(tc.tile_pool(name="recippool", bufs=2))
        rbcpool = ctx.enter_context(tc.tile_pool(name="rbcpool", bufs=2))
        outpool = ctx.enter_context(tc.tile_pool(name="outpool", bufs=2))
        ps_sc = ctx.enter_context(
            tc.tile_pool(name="ps_sc", bufs=2, space="PSUM"))
        ps_ctx = ctx.enter_context(
            tc.tile_pool(name="ps_ctx", bufs=2, space="PSUM"))
        ps_proj = ctx.enter_context(
            tc.tile_pool(name="ps_proj", bufs=2, space="PSUM"))

        # ---- constants (no DMA dependencies) ----
        ones_f32 = singles.tile([1, 128], F32, tag="ones_f32")
        nc.vector.memset(ones_f32, 1.0)
        ones_sb = singles.tile([1, 128], BF16, tag="ones")
        nc.scalar.copy(ones_sb, ones_f32)
        ones_col = singles.tile([128, H, 1], BF16, tag="ones_col")
        nc.vector.memset(ones_col, 1.0)
        # Preload the exp table set while input DMAs stream (first real exp
        # otherwise pays the ~2.7us ACT_TABLE_LOAD mid-pipeline).
        dummy_e = singles.tile([1, 128], F32, tag="dummy_e")
        nc.scalar.activation(dummy_e, ones_f32,
                             mybir.ActivationFunctionType.Exp)

        # ---- DMA schedule ----
        # sync (HWDGE): bvo, wv halves, wq, wk, wo  (+ out stores later)
        # gpsimd (SWDGE): vT, bqk, qT, kT, then eamt per head (+ rbc ops)
        bvo_sb = singles.tile([1, 2 * D], BF16, tag="bvo")
        nc.sync.dma_start(out=bvo_sb, in_=bvo)
        wv_sb = singles.tile([128, 2, NCH * 512], BF16, tag="wv")
        nc.sync.dma_start(out=wv_sb[:, 0, :], in_=wvc[:, 0, :])
        nc.sync.dma_start(out=wv_sb[:, 1, :], in_=wvc[:, 1, :])
        wq_sb = singles.tile([128, NCH, D], BF16, tag="wq")
        nc.sync.dma_start(out=wq_sb, in_=wqc)
        wk_sb = singles.tile([128, NCH, D], BF16, tag="wk")
        nc.sync.dma_start(out=wk_sb, in_=wkc)
        wo_sb = singles.tile([128, NCH, D], BF16, tag="wo")
        nc.sync.dma_start(out=wo_sb, in_=woc)

        vT_sb = singles.tile([128, NCH, S], BF16, tag="vT")
        nc.gpsimd.dma_start(out=vT_sb, in_=vT.rearrange("(c p) s -> p c s", p=128))
        bqk_sb = singles.tile([128, 2 * NCH], F32, tag="bqk")
        nc.gpsimd.dma_start(out=bqk_sb, in_=bqk)
        qT_sb = singles.tile([128, NCH, S], BF16, tag="qT")
        nc.gpsimd.dma_start(out=qT_sb, in_=qT.rearrange("(c p) s -> p c s", p=128))
        kT_sb = singles.tile([128, NCH, S], BF16, tag="kT")
        nc.gpsimd.dma_start(out=kT_sb, in_=kT.rearrange("(c p) s -> p c s", p=128))

        bvr_sb = bvo_sb[:, 0:D]
        bor_sb = bvo_sb[:, D:2 * D]

        # ---- HAM warm-up: junk matmuls while input DMAs stream ----
        for _ in range(36):
            pd = ps_proj.tile([128, 512], F32, tag="proj")
            nc.tensor.matmul(pd[:, :128], lhsT=ones_sb, rhs=ones_sb,
                             start=True, stop=True)

        # persistent activations
        QT_sb = singles.tile([128, NCH, S], BF16, tag="QT")
        KT_sb = singles.tile([128, NCH, S], BF16, tag="KT")
        V_sb = singles.tile([128, SB, H * (DK + 1)], BF16, tag="V")
        ctxT_sb = singles.tile([128, NCH, S], BF16, tag="ctxT")

        # ---- V projection ----
        for sb in range(SB):
            v_heads = V_sb[:, sb, :].rearrange("p (h c) -> p h c", c=DK + 1)
            nc.scalar.copy(v_heads[:, :, DK:DK + 1], ones_col)
        for eh in range(2):
            for sb in range(SB):
                pv = ps_proj.tile([128, 512], F32, tag="proj")
                for dc in range(NCH):
                    nc.tensor.matmul(
                        pv,
                        lhsT=vT_sb[:, dc, sb * 128:(sb + 1) * 128],
                        rhs=wv_sb[:, eh, dc * 512:(dc + 1) * 512],
                        start=(dc == 0), stop=False,
                    )
                nc.tensor.matmul(
                    pv, lhsT=ones_sb,
                    rhs=bvr_sb[:, eh * 512:(eh + 1) * 512],
                    start=False, stop=True,
                )
                v_heads = V_sb[:, sb, :].rearrange("p (h c) -> p h c", c=DK + 1)
                nc.scalar.copy(
                    v_heads[:, 8 * eh:8 * eh + 8, 0:DK],
                    pv.rearrange("p (h d) -> p h d", d=DK),
                )

        # ---- interleaved Q/K projection + attention ----
        eamt_tiles = {}

        def fetch_eamt(h):
            t = eamtpool.tile([128, SB * S], BF16, tag="eamt")
            nc.gpsimd.dma_start(out=t, in_=eamt[h])
            eamt_tiles[h] = t

        fetch_eamt(0)
        fetch_eamt(1)

        def emit_scores_half(h, half):
            """Two score matmuls into one 2-bank psum tile, exp, multiply."""
            i, p0 = h // 2, (h % 2) * 64
            T = ps_sc.tile([128, 1024], F32, tag="sc")
            for kbo in range(2):
                kb = 2 * half + kbo
                nc.tensor.matmul(
                    T[:, kbo * 512:(kbo + 1) * 512],
                    lhsT=KT_sb[p0:p0 + 64, i, kb * 128:(kb + 1) * 128],
                    rhs=QT_sb[p0:p0 + 64, i, :],
                    start=True, stop=True,
                )
            return T

        def emit_exp_mul(h, half, T):
            es = espool.tile([128, 1024], BF16, tag="es")
            nc.scalar.activation(es[:, 0:512], T[:, 0:512],
                                 mybir.ActivationFunctionType.Exp)
            nc.scalar.activation(es[:, 512:1024], T[:, 512:1024],
                                 mybir.ActivationFunctionType.Exp)
            at = attnpool.tile([128, 1024], BF16, tag="at")
            nc.vector.tensor_mul(
                at, es, eamt_tiles[h][:, half * 1024:(half + 1) * 1024])
            return at

        def emit_ctx(h, at_halves):
            pc = ps_ctx.tile([DK + 1, 512], F32, tag="ctx")
            for kb in range(SB):
                nc.tensor.matmul(
                    pc, lhsT=V_sb[:, kb, h * 65:(h + 1) * 65],
                    rhs=at_halves[kb // 2][:, (kb % 2) * 512:(kb % 2 + 1) * 512],
                    start=(kb == 0), stop=(kb == SB - 1),
                )
            return pc

        pend = None      # (pair_idx, pc_a, pc_b, spair) awaiting normalization
        prev_at = None   # (h, at_halves) awaiting ctx matmuls

        def emit_norm(pair_i, pc_a, pc_b, spair):
            rp = recippool.tile([1, 1024], F32, tag="recip")
            nc.vector.reciprocal_approx_fast(out=rp, in_=spair)
            rpb = recippool.tile([1, 1024], BF16, tag="recipb")
            nc.scalar.copy(rpb, rp)
            pr = ps_proj.tile([128, 512], F32, tag="proj")
            nc.tensor.matmul(pr[0:64, :], lhsT=ones_sb[:, 0:64],
                             rhs=rpb[:, 0:512], start=True, stop=True)
            nc.tensor.matmul(pr[64:128, :], lhsT=ones_sb[:, 0:64],
                             rhs=rpb[:, 512:1024], start=True, stop=True)
            rbc = rbcpool.tile([128, 512], F32, tag="rbc")
            nc.scalar.copy(rbc, pr)
            nc.vector.tensor_mul(ctxT_sb[0:64, pair_i, :], pc_a[0:DK, :],
                                 rbc[0:64, :])
            nc.vector.tensor_mul(ctxT_sb[64:128, pair_i, :], pc_b[0:DK, :],
                                 rbc[64:128, :])

        spair_cur = None
        pc_cur = {}

        for i in range(NCH):
            # Q/K projection for chunk i (dense PE work; ACT does psum->SBUF
            # copy with per-partition bias add)
            pq = ps_proj.tile([128, 512], F32, tag="proj")
            for dc in range(NCH):
                nc.tensor.matmul(
                    pq, lhsT=wq_sb[:, i, dc * 128:(dc + 1) * 128],
                    rhs=qT_sb[:, dc, :],
                    start=(dc == 0), stop=(dc == NCH - 1),
                )
            nc.scalar.add(QT_sb[:, i, :], pq, bqk_sb[:, i:i + 1])
            pk = ps_proj.tile([128, 512], F32, tag="proj")
            for dc in range(NCH):
                nc.tensor.matmul(
                    pk, lhsT=wk_sb[:, i, dc * 128:(dc + 1) * 128],
                    rhs=kT_sb[:, dc, :],
                    start=(dc == 0), stop=(dc == NCH - 1),
                )
            nc.scalar.add(KT_sb[:, i, :], pk, bqk_sb[:, NCH + i:NCH + i + 1])

            a, b = 2 * i, 2 * i + 1
            if a + 2 < H:
                fetch_eamt(a + 2)
            if b + 2 < H:
                fetch_eamt(b + 2)

            # software pipeline: fill ACT-exp latency with the previous
            # head's ctx matmuls; norm chain lags by one pair
            Ta0 = emit_scores_half(a, 0)
            at_a0 = emit_exp_mul(a, 0, Ta0)
            Tb0 = emit_scores_half(b, 0)
            at_b0 = emit_exp_mul(b, 0, Tb0)
            if prev_at is not None:
                ph, p_halves = prev_at
                pc = emit_ctx(ph, p_halves)
                pc_cur[ph] = pc
                nc.scalar.copy(
                    spair_cur[:, (ph % 2) * 512:(ph % 2 + 1) * 512],
                    pc[DK:DK + 1, :])
                if ph % 2 == 1:
                    pend = (ph // 2, pc_cur.pop(ph - 1), pc_cur.pop(ph),
                            spair_cur)
            Ta1 = emit_scores_half(a, 1)
            at_a1 = emit_exp_mul(a, 1, Ta1)
            if pend is not None:
                emit_norm(*pend)
                pend = None
            # ctx for head a of THIS pair
            pc = emit_ctx(a, (at_a0, at_a1))
            pc_cur[a] = pc
            spair_cur = sumspool.tile([1, 1024], F32, tag="sums")
            nc.scalar.copy(spair_cur[:, 0:512], pc[DK:DK + 1, :])
            Tb1 = emit_scores_half(b, 1)
            at_b1 = emit_exp_mul(b, 1, Tb1)
            prev_at = (b, (at_b0, at_b1))

        # drain: ctx + norm for the last head / pair
        ph, p_halves = prev_at
        pc = emit_ctx(ph, p_halves)
        pc_cur[ph] = pc
        nc.scalar.copy(spair_cur[:, 512:1024], pc[DK:DK + 1, :])
        emit_norm(ph // 2, pc_cur.pop(ph - 1), pc_cur.pop(ph), spair_cur)

        # ---- output projection ----
        for sb in range(SB):
            for eh in range(2):
                po = ps_proj.tile([128, 512], F32, tag="proj")
                for ch in range(NCH):
                    nc.tensor.matmul(
                        po, lhsT=ctxT_sb[:, ch, sb * 128:(sb + 1) * 128],
                        rhs=wo_sb[:, ch, eh * 512:(eh + 1) * 512],
                        start=(ch == 0), stop=False,
                    )
                nc.tensor.matmul(
                    po, lhsT=ones_sb,
                    rhs=bor_sb[:, eh * 512:(eh + 1) * 512],
                    start=False, stop=True,
                )
                osb = outpool.tile([128, 512], BF16, tag="out")
                nc.scalar.copy(osb, po)
                nc.sync.dma_start(
                    out=out3[sb, :, eh * 512:(eh + 1) * 512], in_=osb)

    nc.compile()
    return nc


def _prep_inputs(query, key, value, mask, w_q, b_q, w_k, b_k, w_v, b_v,
                 w_o, b_o, rel_bias):
    query = np.asarray(query, np.float32)
    key = np.asarray(key, np.float32)
    value = np.asarray(value, np.float32)
    mask = np.asarray(mask)
    w_q = np.asarray(w_q, np.float32)
    w_k = np.asarray(w_k, np.float32)
    w_v = np.asarray(w_v, np.float32)
    w_o = np.asarray(w_o, np.float32)
    b_q = np.asarray(b_q, np.float32)
    b_k = np.asarray(b_k, np.float32)
    b_v = np.asarray(b_v, np.float32)
    b_o = np.asarray(b_o, np.float32)
    rel_bias = np.asarray(rel_bias, np.float32)

    def chunk_w(w):
        # out[p, i, dc*128+m] = w[i*128+m, dc*128+p]
        c = w.reshape(NCH, 128, NCH, 128).transpose(3, 0, 2, 1)
        return np.ascontiguousarray(c).reshape(128, NCH, D).astype(NPBF16)

    wvc = w_v.reshape(2, 512, NCH, 128).transpose(3, 0, 2, 1)
    wvc = np.ascontiguousarray(wvc).reshape(128, 2, NCH * 512).astype(NPBF16)
    bqk = np.concatenate([(b_q / 8.0).reshape(NCH, 128).T,
                          b_k.reshape(NCH, 128).T], axis=1)
    shared = {
        "wqc": chunk_w(w_q / 8.0),
        "wkc": chunk_w(w_k),
        "wvc": wvc,
        "woc": np.ascontiguousarray(
            w_o.T.reshape(NCH, 128, D).transpose(1, 0, 2)).astype(NPBF16),
        "bqk": np.ascontiguousarray(bqk, np.float32),
        "bvo": np.concatenate([b_v, b_o]).reshape(1, 2 * D).astype(NPBF16),
    }

    # ebias[h, k, q] = exp(rel_bias[k - q + 511, h]);  eamt = ebias * mask01
    idx = np.arange(S)[:, None] - np.arange(S)[None, :] + (S - 1)  # [k, q]
    ebias = np.exp(rel_bias[idx])            # [k, q, H]
    ebias = np.ascontiguousarray(ebias.transpose(2, 0, 1))  # [H, k, q]

    in_maps = []
    for c in range(N_CORES):
        m01 = (mask[c, 0].T != 0).astype(np.float32)     # [k, q]
        ea = (ebias * m01[None]).astype(NPBF16)          # [H, k, q]
        ea = ea.reshape(H, SB, 128, S).transpose(0, 2, 1, 3)
        ea = np.ascontiguousarray(ea).reshape(H, 128, SB * S)
        im = dict(shared)
        im["qT"] = np.ascontiguousarray(query[c].T).astype(NPBF16)
        im["kT"] = np.ascontiguousarray(key[c].T).astype(NPBF16)
        im["vT"] = np.ascontiguousarray(value[c].T).astype(NPBF16)
        im["eamt"] = ea
        in_maps.append(im)
    return in_maps


def kernel(query, key, value, mask, w_q, b_q, w_k, b_k, w_v, b_v, w_o, b_o,
           rel_bias, _run_opts=None):
    if "nc" not in _CACHE:
        _CACHE["nc"] = _build_program()
    nc = _CACHE["nc"]
    in_maps = _prep_inputs(query, key, value, mask, w_q, b_q, w_k, b_k,
                           w_v, b_v, w_o, b_o, rel_bias)
    opts = _run_opts or {}
    res = run_bass_kernel_spmd(nc, in_maps, list(range(N_CORES)), **opts)
    out = np.stack([np.asarray(res.results[c]["out"]) for c in range(N_CORES)])
    if _run_opts is not None:
        _CACHE["last_result"] = res
    return out.astype(np.float32)


# revision 14
# speedup vs baseline: 1.6025x; 1.1987x over previous
"""MultiHeadAttention Trainium2 kernel (8 NeuronCores, data-parallel over batch).

Contract: kernel(**inputs) takes the FULL inputs from setup_inputs() and
returns the FULL [8, 512, 1024] output. Batch element c runs on NeuronCore c
(B == n_cores == 8); each core runs the same Bass/Tile program on its own
shard. No collectives.

All matmuls run in bf16 (full-rate 1 elem/cycle PE streaming; fp32(r) streams
at half rate on TRN2) with fp32 PSUM accumulation. The additive rel-bias +
mask is applied multiplicatively after the exp:
    exp(QK/8 + bias + maskadd) = exp(QK/8) * eamt,   eamt = exp(bias)*mask01
eamt is precomputed on the host in bf16, so the scores path is
    PE matmul -> ACT exp (one op per [128,1024] two-bank psum tile)
    -> DVE bf16 multiply (2x mode)
with no slow fp32-PSUM-side DVE add. Softmax denominators come from a ones
column appended to V per head (row 64 of the ctx psum); reciprocals are
broadcast across partitions with the GpSimd partition_broadcast ucode op.

Per-core computation (batch b, S=512, D=1024, H=16, Dk=64):
  QT = (w_q/8)-proj of query^T  -> [D, S] bf16   (head h rows h*64..h*64+63)
  KT likewise (unscaled)        -> [D, S] bf16
  V  = natural value proj       -> [S, D] bf16 (+ ones column per head)
  per head: scoresT[k,q] in psum; attn = exp(scoresT)*eamt[h]
            ctxT[65,S] = [V_h | 1]^T @ attn ; ctxT_norm = ctxT[0:64] * (1/row64)
  out[q,e] = ctxT_norm^T-chunks @ w_o^T + b_o    (bf16, fp32 psum)
"""
import numpy as np
import ml_dtypes

import concourse.bass as bass
import concourse.tile as tile
from concourse import bacc, mybir
from concourse.bass_utils import run_bass_kernel_spmd

S = 512
D = 1024
H = 16
DK = 64
N_CORES = 8
NCH = D // 128  # 8 d-model chunks of 128
SB = S // 128   # 4 seq blocks of 128
F32 = mybir.dt.float32
BF16 = mybir.dt.bfloat16
NPBF16 = ml_dtypes.bfloat16

MASK_NEG = -30000.0

_CACHE = {}


def _build_program():
    nc = bacc.Bacc("TRN2", target_bir_lowering=False, debug=False,
                   num_devices=N_CORES)

    # Per-core DRAM inputs (qT/kT/vT already in [128, chunk, s] layout)
    qT = nc.dram_tensor("qT", [128, NCH, S], BF16, kind="ExternalInput").ap()
    kT = nc.dram_tensor("kT", [128, NCH, S], BF16, kind="ExternalInput").ap()
    vT = nc.dram_tensor("vT", [128, NCH, S], BF16, kind="ExternalInput").ap()
    eamt = nc.dram_tensor("eamt", [H, 128, SB * S], BF16,
                          kind="ExternalInput").ap()
    wqc = nc.dram_tensor("wqc", [128, NCH, D], BF16, kind="ExternalInput").ap()
    wkc = nc.dram_tensor("wkc", [128, NCH, D], BF16, kind="ExternalInput").ap()
    wvc = nc.dram_tensor("wvc", [128, 2, NCH * 512], BF16,
                         kind="ExternalInput").ap()
    woc = nc.dram_tensor("woc", [128, NCH, D], BF16, kind="ExternalInput").ap()
    bqk = nc.dram_tensor("bqk", [128, 2 * NCH], F32, kind="ExternalInput").ap()
    bvo = nc.dram_tensor("bvo", [1, 2 * D], BF16, kind="ExternalInput").ap()
    out = nc.dram_tensor("out", [S, D], BF16, kind="ExternalOutput").ap()

    out3 = out.rearrange("(sb p) e -> sb p e", p=128)  # [4, 128, 1024]

    from contextlib import ExitStack

    with tile.TileContext(nc) as tc, ExitStack() as ctx:
        singles = ctx.enter_context(tc.tile_pool(name="singles", bufs=1))
        eamtpool = ctx.enter_context(tc.tile_pool(name="eamtpool", bufs=4))
        espool = ctx.enter_context(tc.tile_pool(name="espool", bufs=3))
        attnpool = ctx.enter_context(tc.tile_pool(name="attnpool", bufs=4))
        recippool = ctx.enter_context(tc.tile_pool(name="recippool", bufs=2))
        rbcpool = ctx.enter_context(tc.tile_pool(name="rbcpool", bufs=2))
        outpool = ctx.enter_context(tc.tile_pool(name="outpool", bufs=2))
        ps_sc = ctx.enter_context(
            tc.tile_pool(name="ps_sc", bufs=2, space="PSUM"))
        ps_ctx = ctx.enter_context(
            tc.tile_pool(name="ps_ctx", bufs=2, space="PSUM"))
        ps_proj = ctx.enter_context(
            tc.tile_pool(name="ps_proj", bufs=2, space="PSUM"))

        # ---- DMA schedule ----
        # sync (HWDGE): wv half 0, vT, wv half 1, wq, wk, wo (+ out stores)
        # scalar (HWDGE, separate ring): bvo, qT, kT, bqk
        # gpsimd (SWDGE): eamt per head only
        wv_sb = singles.tile([128, 2, NCH * 512], BF16, tag="wv")
        vT_sb = singles.tile([128, NCH, S], BF16, tag="vT")
        nc.sync.dma_start(out=wv_sb[:, 0, :], in_=wvc[:, 0, :])
        nc.sync.dma_start(out=vT_sb, in_=vT)
        nc.sync.dma_start(out=wv_sb[:, 1, :], in_=wvc[:, 1, :])
        wq_sb = singles.tile([128, NCH, D], BF16, tag="wq")
        nc.sync.dma_start(out=wq_sb, in_=wqc)
        wk_sb = singles.tile([128, NCH, D], BF16, tag="wk")
        nc.sync.dma_start(out=wk_sb, in_=wkc)
        wo_sb = singles.tile([128, NCH, D], BF16, tag="wo")
        nc.sync.dma_start(out=wo_sb, in_=woc)

        bvo_sb = singles.tile([1, 2 * D], BF16, tag="bvo")
        nc.scalar.dma_start(out=bvo_sb, in_=bvo)
        qT_sb = singles.tile([128, NCH, S], BF16, tag="qT")
        nc.scalar.dma_start(out=qT_sb, in_=qT)
        kT_sb = singles.tile([128, NCH, S], BF16, tag="kT")
        nc.scalar.dma_start(out=kT_sb, in_=kT)
        bqk_sb = singles.tile([128, 2 * NCH], F32, tag="bqk")
        nc.scalar.dma_start(out=bqk_sb, in_=bqk)

        bvr_sb = bvo_sb[:, 0:D]
        bor_sb = bvo_sb[:, D:2 * D]

        # ---- constants ----
        ones_f32 = singles.tile([1, 128], F32, tag="ones_f32")
        nc.vector.memset(ones_f32, 1.0)
        ones_sb = singles.tile([1, 128], BF16, tag="ones")
        nc.scalar.copy(ones_sb, ones_f32)
        allones = singles.tile([128, DK], BF16, tag="allones")
        nc.vector.memset(allones, 1.0)
        # Preload the exp table set while input DMAs stream (first real exp
        # otherwise pays the ~2.7us ACT_TABLE_LOAD mid-pipeline).
        dummy_e = singles.tile([1, 128], F32, tag="dummy_e")
        nc.scalar.activation(dummy_e, ones_f32,
                             mybir.ActivationFunctionType.Exp)

        # ---- HAM warm-up: junk matmuls while input DMAs stream ----
        for _ in range(36):
            pd = ps_proj.tile([128, 512], F32, tag="proj")
            nc.tensor.matmul(pd[:, :128], lhsT=ones_sb, rhs=ones_sb,
                             start=True, stop=True)

        # persistent activations
        QT_sb = singles.tile([128, NCH, S], BF16, tag="QT")
        KT_sb = singles.tile([128, NCH, S], BF16, tag="KT")
        V_sb = singles.tile([128, SB, H * DK], BF16, tag="V")
        ctxT_sb = singles.tile([128, NCH, S], BF16, tag="ctxT")

        # ---- V projection ----
        for eh in range(2):
            for sb in range(SB):
                pv = ps_proj.tile([128, 512], F32, tag="proj")
                for dc in range(NCH):
                    nc.tensor.matmul(
                        pv,
                        lhsT=vT_sb[:, dc, sb * 128:(sb + 1) * 128],
                        rhs=wv_sb[:, eh, dc * 512:(dc + 1) * 512],
                        start=(dc == 0), stop=False,
                    )
                nc.tensor.matmul(
                    pv, lhsT=ones_sb,
                    rhs=bvr_sb[:, eh * 512:(eh + 1) * 512],
                    start=False, stop=True,
                )
                nc.scalar.copy(V_sb[:, sb, eh * 512:(eh + 1) * 512], pv)

        # ---- interleaved Q/K projection + attention ----
        eamt_tiles = {}

        def fetch_eamt(h):
            t = eamtpool.tile([128, SB * S], BF16, tag="eamt")
            nc.gpsimd.dma_start(out=t, in_=eamt[h])
            eamt_tiles[h] = t

        fetch_eamt(0)
        fetch_eamt(1)

        def emit_scores_half(h, half):
            """Two score matmuls into one 2-bank psum tile, exp, multiply."""
            i, p0 = h // 2, (h % 2) * 64
            T = ps_sc.tile([128, 1024], F32, tag="sc")
            for kbo in range(2):
                kb = 2 * half + kbo
                nc.tensor.matmul(
                    T[:, kbo * 512:(kbo + 1) * 512],
                    lhsT=KT_sb[p0:p0 + 64, i, kb * 128:(kb + 1) * 128],
                    rhs=QT_sb[p0:p0 + 64, i, :],
                    start=True, stop=True,
                )
            return T

        def emit_exp_mul(h, half, T):
            es = espool.tile([128, 1024], BF16, tag="es")
            nc.scalar.activation(es, T, mybir.ActivationFunctionType.Exp)
            at = attnpool.tile([128, 1024], BF16, tag="at")
            nc.vector.tensor_mul(
                at, es, eamt_tiles[h][:, half * 1024:(half + 1) * 1024])
            return at

        def emit_ctx(h, at_halves):
            """One 8-matmul accumulation group in one psum bank: rows 0-63 =
            ctx_h, rows 64-127 = denominator replicated via all-ones lhsT.
            Only the first matmul clears the bank's has_written bits."""
            pc = ps_ctx.tile([128, 512], F32, tag="ctx")
            for kb in range(SB):
                nc.tensor.matmul(
                    pc[0:DK, :], lhsT=V_sb[:, kb, h * DK:(h + 1) * DK],
                    rhs=at_halves[kb // 2][:, (kb % 2) * 512:(kb % 2 + 1) * 512],
                    start=(kb == 0), stop=False, skip_group_check=True,
                )
            for kb in range(SB):
                nc.tensor.matmul(
                    pc[DK:128, :], lhsT=allones,
                    rhs=at_halves[kb // 2][:, (kb % 2) * 512:(kb % 2 + 1) * 512],
                    start=(kb == 0), stop=(kb == SB - 1),
                    skip_group_check=True,
                )
            return pc

        def emit_norm_h(h, pc):
            den = recippool.tile([DK, 512], F32, tag="den")
            nc.vector.tensor_copy(den, pc[DK:128, :])
            rec = rbcpool.tile([DK, 512], F32, tag="rec")
            nc.vector.reciprocal_approx_fast(out=rec, in_=den)
            i, p0 = h // 2, (h % 2) * 64
            nc.vector.tensor_mul(ctxT_sb[p0:p0 + 64, i, :], pc[0:DK, :], rec)

        prev_at = None   # (h, at_halves) awaiting ctx+den matmuls

        for i in range(NCH):
            # K then Q projection for chunk i; ACT does the psum->SBUF copy
            # with per-partition bias add
            pk = ps_proj.tile([128, 512], F32, tag="proj")
            for dc in range(NCH):
                nc.tensor.matmul(
                    pk, lhsT=wk_sb[:, i, dc * 128:(dc + 1) * 128],
                    rhs=kT_sb[:, dc, :],
                    start=(dc == 0), stop=(dc == NCH - 1),
                )
            nc.scalar.add(KT_sb[:, i, :], pk, bqk_sb[:, NCH + i:NCH + i + 1])
            pq = ps_proj.tile([128, 512], F32, tag="proj")
            for dc in range(NCH):
                nc.tensor.matmul(
                    pq, lhsT=wq_sb[:, i, dc * 128:(dc + 1) * 128],
                    rhs=qT_sb[:, dc, :],
                    start=(dc == 0), stop=(dc == NCH - 1),
                )
            nc.scalar.add(QT_sb[:, i, :], pq, bqk_sb[:, i:i + 1])

            a, b = 2 * i, 2 * i + 1
            if a + 2 < H:
                fetch_eamt(a + 2)
            if b + 2 < H:
                fetch_eamt(b + 2)

            # previous head's ctx+den matmuls fill the QT/KT-add latency
            if prev_at is not None:
                ph, p_halves = prev_at
                pc = emit_ctx(ph, p_halves)
                emit_norm_h(ph, pc)
            Ta0 = emit_scores_half(a, 0)
            at_a0 = emit_exp_mul(a, 0, Ta0)
            Tb0 = emit_scores_half(b, 0)
            at_b0 = emit_exp_mul(b, 0, Tb0)
            Ta1 = emit_scores_half(a, 1)
            at_a1 = emit_exp_mul(a, 1, Ta1)
            Tb1 = emit_scores_half(b, 1)
            at_b1 = emit_exp_mul(b, 1, Tb1)
            pc = emit_ctx(a, (at_a0, at_a1))
            emit_norm_h(a, pc)
            prev_at = (b, (at_b0, at_b1))

        ph, p_halves = prev_at
        pc = emit_ctx(ph, p_halves)
        emit_norm_h(ph, pc)

        # ---- output projection ----
        for sb in range(SB):
            for eh in range(2):
                po = ps_proj.tile([128, 512], F32, tag="proj")
                for ch in range(NCH):
                    nc.tensor.matmul(
                        po, lhsT=ctxT_sb[:, ch, sb * 128:(sb + 1) * 128],
                        rhs=wo_sb[:, ch, eh * 512:(eh + 1) * 512],
                        start=(ch == 0), stop=False,
                    )
                nc.tensor.matmul(
                    po, lhsT=ones_sb,
                    rhs=bor_sb[:, eh * 512:(eh + 1) * 512],
                    start=False, stop=True,
                )
                osb = outpool.tile([128, 512], BF16, tag="out")
                nc.scalar.copy(osb, po)
                nc.sync.dma_start(
                    out=out3[sb, :, eh * 512:(eh + 1) * 512], in_=osb)

    nc.compile()
    return nc


def _prep_inputs(query, key, value, mask, w_q, b_q, w_k, b_k, w_v, b_v,
                 w_o, b_o, rel_bias):
    query = np.asarray(query, np.float32)
    key = np.asarray(key, np.float32)
    value = np.asarray(value, np.float32)
    mask = np.asarray(mask)
    w_q = np.asarray(w_q, np.float32)
    w_k = np.asarray(w_k, np.float32)
    w_v = np.asarray(w_v, np.float32)
    w_o = np.asarray(w_o, np.float32)
    b_q = np.asarray(b_q, np.float32)
    b_k = np.asarray(b_k, np.float32)
    b_v = np.asarray(b_v, np.float32)
    b_o = np.asarray(b_o, np.float32)
    rel_bias = np.asarray(rel_bias, np.float32)

    def chunk_w(w):
        # out[p, i, dc*128+m] = w[i*128+m, dc*128+p]
        c = w.reshape(NCH, 128, NCH, 128).transpose(3, 0, 2, 1)
        return np.ascontiguousarray(c).reshape(128, NCH, D).astype(NPBF16)

    wvc = w_v.reshape(2, 512, NCH, 128).transpose(3, 0, 2, 1)
    wvc = np.ascontiguousarray(wvc).reshape(128, 2, NCH * 512).astype(NPBF16)
    bqk = np.concatenate([(b_q / 8.0).reshape(NCH, 128).T,
                          b_k.reshape(NCH, 128).T], axis=1)
    shared = {
        "wqc": chunk_w(w_q / 8.0),
        "wkc": chunk_w(w_k),
        "wvc": wvc,
        "woc": np.ascontiguousarray(
            w_o.T.reshape(NCH, 128, D).transpose(1, 0, 2)).astype(NPBF16),
        "bqk": np.ascontiguousarray(bqk, np.float32),
        "bvo": np.concatenate([b_v, b_o]).reshape(1, 2 * D).astype(NPBF16),
    }

    # ebias[h, k, q] = exp(rel_bias[k - q + 511, h]);  eamt = ebias * mask01
    idx = np.arange(S)[:, None] - np.arange(S)[None, :] + (S - 1)  # [k, q]
    ebias = np.exp(rel_bias[idx])            # [k, q, H]
    ebias = np.ascontiguousarray(ebias.transpose(2, 0, 1))  # [H, k, q]

    in_maps = []
    for c in range(N_CORES):
        m01 = (mask[c, 0].T != 0).astype(np.float32)     # [k, q]
        ea = (ebias * m01[None]).astype(NPBF16)          # [H, k, q]
        ea = ea.reshape(H, SB, 128, S).transpose(0, 2, 1, 3)
        ea = np.ascontiguousarray(ea).reshape(H, 128, SB * S)
        im = dict(shared)
        def pcs(x):
            # [S, D] -> xT [D, S] -> [128, NCH, S] chunk layout
            t = x.T.reshape(NCH, 128, S).transpose(1, 0, 2)
            return np.ascontiguousarray(t).astype(NPBF16)

        im["qT"] = pcs(query[c])
        im["kT"] = pcs(key[c])
        im["vT"] = pcs(value[c])
        im["eamt"] = ea
        in_maps.append(im)
    return in_maps


def kernel(query, key, value, mask, w_q, b_q, w_k, b_k, w_v, b_v, w_o, b_o,
           rel_bias, _run_opts=None):
    if "nc" not in _CACHE:
        _CACHE["nc"] = _build_program()
    nc = _CACHE["nc"]
    in_maps = _prep_inputs(query, key, value, mask, w_q, b_q, w_k, b_k,
                           w_v, b_v, w_o, b_o, rel_bias)
    opts = _run_opts or {}
    res = run_bass_kernel_spmd(nc, in_maps, list(range(N_CORES)), **opts)
    out = np.stack([np.asarray(res.results[c]["out"]) for c in range(N_CORES)])
    if _run_opts is not None:
        _CACHE["last_result"] = res
    return out.astype(np.float32)
